# revision 14
# baseline (speedup 1.0000x reference)
"""SpGAT_Conv Trainium2 kernel: 8-core SPMD spectral GNN conv.

Math (reference):
    a = softmax(alpha)
    pre = x @ W                                   [N, D]
    out_low  = s0 @ (a0 * (s1 @ pre))             [N, D]
    out_high = s2 @ (a1 * (s3 @ pre))             [N, D]
    out = relu(max(out_low, out_high) + bias)

Re-association: t = S @ (x @ W) == (S @ x) @ W with S = concat(s1, s3).
Row-sharding t's rows across 8 cores makes the x@W work perfectly sharded
too (it rides on each core's own 1024 rows of u = S_c @ x) instead of being
replicated, cutting per-core PE work from 1280 to 1056 big matmuls:

    step 1: u_c^T = x^T S_c^T accumulated over n-chunks; stationary = x
            chunks (natural layout), moving = S_c^T strips.  Two sweeps of
            512 i-columns each (4 PSUM banks per sweep).
    step 2: t_c = u_c @ W via u^T slices stationary, W moving (32 matmuls);
            each sweep's 512 t rows are staged + AllGathered in two
            sub-collectives (4 total, order 0,1,3,2) while later compute
            runs.
    phase 3: out_c = relu(max(a0*s0_c@t1 + bias, a1*s2_c@t3 + bias)), high
             band then low band over ARRIVAL-ordered t chunks; PSUM is
             pre-seeded with bias/a so no separate bias add exists in the
             epilogue.

DMA discipline: DMA_DIRECT2D executes synchronously on the issuing
engine's queue AND all queues share one small DMA-completion semaphore
pool, so a blocked DMA (or a semaphore-reset rendezvous with one) stalls
unrelated queues.  Mitigations:
  - Sync queue: only matmul-pacing strip loads, t_in stores, half the out
    stores — never anything collective-gated.
  - Activation queue: bulk x/w loads (head), AllGather-gated t_sb loads
    (tail), half the out stores.
  - Batched 3D-AP transfers minimize DMA instruction count (≈92 total) so
    semaphore epochs recycle rarely.
  - Vector/GpSimd split all PSUM drain/seed/stash/epilogue work.

All big operands are host-cast to bf16 (full PE rate) with fp32 PSUM
accumulation; s-matrices are host-transposed so the contraction dim lands
on SBUF partitions with contiguous DMAs.  x needs no transpose in this
formulation.
"""

import os

import numpy as np

N_CORES = 8
N = 8192
K = 2048
NK = N - K          # 6144
D = 512
ROWS = N // N_CORES  # 1024 rows per core
P = 128
RCH = ROWS // P      # 8  (row chunks per core / output strips)
NCH = N // P         # 64 (contraction chunks over full N)
KCH = K // P         # 16 (low-band chunks; high band = NCH - KCH = 48)
DCH = D // P         # 4  (depth chunks)
NSUB = 4             # sub-AllGathers for t
SUBR = ROWS // NSUB  # 256 rows per rank per sub-AG
SB = 4               # n-chunks per sweep-strip DMA batch

DEBUG = os.environ.get("SPGAT_DEBUG", "0") == "1"

_CACHE = {}

# t-chunk arrival order: sub-AG g delivers, for every rank c, t rows
# [1024c + 256g, 1024c + 256(g+1)) = global chunks 8c + 2g + {0,1}.
# Gathers are triggered in order 0, 1, 3, 2 (sweep B stages its upper rows
# first), so consume in that order too.  Phase 3 iterates PAIRS (both u's
# of one (g, c)) so each pair is one strip DMA.
GORDER = [0, 1, 3, 2]
PAIRS = [(8 * c + 2 * g, g, c) for g in GORDER for c in range(N_CORES)]


def _build_nc():
    import concourse.mybir as mybir
    import concourse.tile as tile
    from concourse import bacc

    f32 = mybir.dt.float32
    bf16 = mybir.dt.bfloat16
    cdt = bf16

    nc = bacc.Bacc(
        "TRN2", target_bir_lowering=False, debug=False, num_devices=N_CORES
    )

    x = nc.dram_tensor("x", [N, D], cdt, kind="ExternalInput").ap()
    w = nc.dram_tensor("w", [D, D], cdt, kind="ExternalInput").ap()
    alpha = nc.dram_tensor("alpha", [2], f32, kind="ExternalInput").ap()
    bias = nc.dram_tensor("bias", [D], f32, kind="ExternalInput").ap()
    st = nc.dram_tensor("st", [N, ROWS], cdt, kind="ExternalInput").ap()
    s0t = nc.dram_tensor("s0t", [K, ROWS], cdt, kind="ExternalInput").ap()
    s2t = nc.dram_tensor("s2t", [NK, ROWS], cdt, kind="ExternalInput").ap()
    out = nc.dram_tensor("out", [ROWS, D], f32, kind="ExternalOutput").ap()
    if DEBUG:
        t_dump = nc.dram_tensor("t_dump", [N, D], cdt, kind="ExternalOutput").ap()

    groups = [list(range(N_CORES))]

    with tile.TileContext(nc) as tc:
        with (
            tc.tile_pool(name="const", bufs=1) as const,
            tc.tile_pool(name="bigA", bufs=1) as bigA,
            tc.tile_pool(name="bigB", bufs=1) as bigB,
            tc.tile_pool(name="strips1", bufs=3) as strips1,
            tc.tile_pool(name="strips3", bufs=3) as strips3,
            tc.tile_pool(name="stage", bufs=4) as stage,
            tc.tile_pool(name="epi", bufs=2) as epi,
            tc.tile_pool(name="stash", bufs=1) as stashp,
            tc.tile_pool(name="ps", bufs=8, space="PSUM") as ps,
            tc.tile_pool(name="dram", bufs=1, space="DRAM") as dram,
        ):
            # ---- collective warm-up: absorb first-collective init.  No
            # input deps so the trigger is the very first gpsimd instruction.
            warm_in = dram.tile([8, 8], f32, name="warm_in")
            warm_out = dram.tile([64, 8], f32, name="warm_out", addr_space="Shared")
            nc.gpsimd.collective_compute(
                "AllGather",
                mybir.AluOpType.bypass,
                replica_groups=groups,
                ins=[warm_in.opt()],
                outs=[warm_out.opt()],
            )

            # ---- PSUM allocation order fixes the 8-bank rotation; all
            # tiles are one 2KB bank.
            ps_a = ps.tile([P, 2], f32, name="ps_a", tag="acc")
            ps_b0 = ps.tile([P, D], f32, name="ps_b0", tag="acc")
            ps_b1 = ps.tile([P, D], f32, name="ps_b1", tag="acc")
            accsA = [ps.tile([P, D], f32, name=f"uA_{dc}", tag="acc")
                     for dc in range(DCH)]
            accsB = [ps.tile([P, D], f32, name=f"uB_{dc}", tag="acc")
                     for dc in range(DCH)]

            # ---- input DMAs: alpha/bias tiny on sync; x/w bulk on the
            # scalar (Activation) HWDGE queue so the sync queue only ever
            # carries matmul-pacing strip loads.
            asb = const.tile([1, 2], f32, name="asb")
            nc.sync.dma_start(asb[:], alpha[None, :])
            bsb = const.tile([1, D], f32, name="bsb")
            nc.sync.dma_start(bsb[:], bias[None, :])

            x_sb = bigA.tile([P, NCH, D], cdt, name="x_sb", tag="bigA")
            x_v = x.rearrange("(c p) d -> p c d", p=P)
            # small leading blocks so the first step-1 matmuls start fast,
            # then three bulk transfers
            for c0, c1 in ((0, 1), (1, 2), (2, 4), (4, 24), (24, 44), (44, 64)):
                nc.scalar.dma_start(x_sb[:, c0:c1, :], x_v[:, c0:c1, :])
            w_sb = const.tile([P, DCH, D], cdt, name="w_sb")
            nc.scalar.dma_start(w_sb[:], w.rearrange("(c p) d -> p c d", p=P))

            # ---- steps 1+2 infrastructure
            t_in = dram.tile([ROWS, D], cdt, name="t_in")
            t_outs = [
                dram.tile([SUBR * N_CORES, D], cdt, name=f"t_out{g}",
                          addr_space="Shared")
                for g in range(NSUB)
            ]

            def t_subag(g):
                nc.gpsimd.collective_compute(
                    "AllGather",
                    mybir.AluOpType.bypass,
                    replica_groups=groups,
                    ins=[t_in[SUBR * g : SUBR * (g + 1), :].opt()],
                    outs=[t_outs[g].opt()],
                )

            ut_sb = [
                const.tile([P, DCH, D], cdt, name=f"ut{sw}") for sw in range(2)
            ]

            st_v = st.rearrange("(c p) i -> p c i", p=P)

            def sweep1(sw, accs, b0, b1, split_first=False):
                """step-1 matmuls for strip batches [b0, b1) of sweep sw;
                each batch is SB n-chunks in one DMA."""
                col0 = D * sw
                for bk in range(b0, b1):
                    j0 = SB * bk
                    strip = strips1.tile([P, SB, D], cdt, name=f"s{sw}_{bk}",
                                         tag="strip")
                    if split_first and bk == b0:
                        for j2 in range(SB):
                            nc.sync.dma_start(
                                strip[:, j2 : j2 + 1, :],
                                st_v[:, j0 + j2 : j0 + j2 + 1,
                                     col0 : col0 + D],
                            )
                    else:
                        nc.sync.dma_start(
                            strip[:], st_v[:, j0 : j0 + SB, col0 : col0 + D]
                        )
                    for j2 in range(SB):
                        j = j0 + j2
                        for dc in range(DCH):
                            nc.tensor.matmul(
                                accs[dc][:],
                                x_sb[:, j, P * dc : P * (dc + 1)],
                                strip[:, j2, :],
                                start=(j == 0),
                                stop=(j == NCH - 1),
                            )

            def drain_u(sw, accs):
                for dc in range(DCH):
                    nc.vector.tensor_copy(ut_sb[sw][:, dc, :], accs[dc][:])

            def step2(sw, tps=None):
                """t rows [512sw, 512sw+512): 4 i-blocks of 128 rows; stage
                + trigger this sweep's two sub-AGs (B stages upper first)."""
                ib_order = [0, 1, 2, 3] if sw == 0 else [2, 3, 0, 1]
                ag_map = {1: 0, 3: 1} if sw == 0 else {1: 3, 3: 2}
                for i, ib in enumerate(ib_order):
                    if tps is None:
                        tp = ps.tile([P, D], f32, name=f"t_ps_{sw}_{ib}",
                                     tag="acc")
                    else:
                        tp = tps[i]
                    for dc in range(DCH):
                        nc.tensor.matmul(
                            tp[:],
                            ut_sb[sw][:, dc, P * ib : P * (ib + 1)],
                            w_sb[:, dc, :],
                            start=(dc == 0),
                            stop=(dc == DCH - 1),
                        )
                    tst = stage.tile([P, D], cdt, name=f"t_st_{sw}_{ib}",
                                     tag="st")
                    nc.vector.tensor_copy(tst[:], tp[:])
                    row0 = D * sw + P * ib
                    nc.sync.dma_start(t_in[row0 : row0 + P, :], tst[:])
                    if i in ag_map:
                        t_subag(ag_map[i])

            NB = NCH // SB  # 16 strip batches per sweep

            # ---- PE stream head: first strip batch goes before the softmax
            # setup matmuls so the PE starts as soon as chunk 0 lands.
            sweep1(0, accsA, 0, 1, split_first=True)

            # softmax(alpha) chain (vector) + broadcasts via tiny matmuls
            amax = const.tile([1, 1], f32, name="amax")
            nc.vector.tensor_tensor(
                amax[:], asb[:, 0:1], asb[:, 1:2], mybir.AluOpType.max
            )
            ash = const.tile([1, 2], f32, name="ash")
            nc.vector.tensor_scalar(
                ash[:], asb[:], amax[:, 0:1], None, mybir.AluOpType.subtract
            )
            aexp = const.tile([1, 2], f32, name="aexp")
            nc.scalar.activation(aexp[:], ash[:], mybir.ActivationFunctionType.Exp)
            asum = const.tile([1, 1], f32, name="asum")
            nc.vector.tensor_tensor(
                asum[:], aexp[:, 0:1], aexp[:, 1:2], mybir.AluOpType.add
            )
            arec = const.tile([1, 1], f32, name="arec")
            nc.vector.reciprocal(arec[:], asum[:])
            afin = const.tile([1, 2], f32, name="afin")
            nc.vector.tensor_scalar(
                afin[:], aexp[:], arec[:, 0:1], None, mybir.AluOpType.mult
            )
            ainv = const.tile([1, 2], f32, name="ainv")
            nc.vector.reciprocal(ainv[:], afin[:])
            srow0 = const.tile([1, D], f32, name="srow0")
            nc.vector.tensor_scalar(
                srow0[:], bsb[:], ainv[:, 0:1], None, mybir.AluOpType.mult
            )
            srow1 = const.tile([1, D], f32, name="srow1")
            nc.vector.tensor_scalar(
                srow1[:], bsb[:], ainv[:, 1:2], None, mybir.AluOpType.mult
            )
            ones = const.tile([1, P], f32, name="ones")
            nc.vector.memset(ones[:], 1.0)
            nc.tensor.matmul(ps_a[:], ones[:], afin[:], start=True, stop=True)
            a128 = const.tile([P, 2], f32, name="a128")
            nc.vector.tensor_copy(a128[:], ps_a[:])
            seeds = const.tile([P, 2 * D], f32, name="seeds")
            nc.tensor.matmul(ps_b0[:], ones[:], srow0[:], start=True, stop=True)
            nc.vector.tensor_copy(seeds[:, 0:D], ps_b0[:])
            nc.tensor.matmul(ps_b1[:], ones[:], srow1[:], start=True, stop=True)
            nc.vector.tensor_copy(seeds[:, D : 2 * D], ps_b1[:])
            seed_lo = seeds[:, 0:D]
            seed_hi = seeds[:, D : 2 * D]

            # ---- rest of sweep A, then pipelined B/step2
            sweep1(0, accsA, 1, NB)
            drain_u(0, accsA)
            # head of sweep B hides sweep A's drain latency before step2A
            sweep1(1, accsB, 0, 2)
            step2(0)
            sweep1(1, accsB, 2, NB)
            drain_u(1, accsB)

            # PSUM rotation: allocate step2B's tiles, then phase-3
            # accumulators — accs3[0..3] land on step2A's banks (free
            # early), accs3[4..7] on step2B's.  Seed 0..3 before step2B's
            # instructions so only 4..7 wait on its staging.
            tps1 = [ps.tile([P, D], f32, name=f"t_ps_1_{i}", tag="acc")
                    for i in range(4)]
            accs3 = [
                ps.tile([P, D], f32, name=f"acc3_{nt}", tag="acc")
                for nt in range(RCH)
            ]
            for nt in range(4):
                nc.vector.tensor_copy(accs3[nt][:], seed_hi)
            step2(1, tps1)
            for nt in range(4, RCH):
                nc.scalar.copy(accs3[nt][:], seed_hi)

            # ---- phase 3: t gather consumption + band matmuls.  One t_sb
            # load per (sub-AG, u): 8 gather DMAs total on the scalar queue.
            t_sb = bigB.tile([P, NCH, D], cdt, name="t_sb", tag="bigB")
            t_sb_r = t_sb[:].rearrange("p (c r) d -> p r c d", r=8)
            for g in GORDER:
                for u in range(2):
                    # chunks j = 8c + 2g + u for c in 0..7
                    nc.scalar.dma_start(
                        t_sb_r[:, 2 * g + u, :, :],
                        t_outs[g].rearrange("(c q p) d -> p c q d", p=P, q=2)[
                            :, :, u, :
                        ],
                    )
            if DEBUG:
                for j in range(NCH):
                    nc.sync.dma_start(
                        t_dump[P * j : P * (j + 1), :], t_sb[:, j, :]
                    )

            HI_PAIRS = [e for e in PAIRS if e[0] >= KCH]
            LO_PAIRS = [e for e in PAIRS if e[0] < KCH]
            stash = [
                stashp.tile([P, D], f32, name=f"hst_{nt}", tag=f"hst{nt}")
                for nt in range(RCH)
            ]
            s2t_v = s2t.rearrange("(c p) i -> p c i", p=P)
            s0t_v = s0t.rearrange("(c p) i -> p c i", p=P)
            for idx, (j, g, c) in enumerate(HI_PAIRS):
                jj = j - KCH
                strip = strips3.tile([P, 2, ROWS], cdt, name=f"rh_{j}",
                                     tag="strip3")
                nc.sync.dma_start(strip[:], s2t_v[:, jj : jj + 2, :])
                for u in range(2):
                    for nt in range(RCH):
                        nc.tensor.matmul(
                            accs3[nt][:],
                            strip[:, u, P * nt : P * (nt + 1)],
                            t_sb[:, j + u, :],
                            start=False,
                            stop=(idx == len(HI_PAIRS) - 1 and u == 1),
                        )
            # stash = a1*acc = a1*hi + bias, then re-seed for the low band
            # right behind the stash read; split vector/scalar (gpsimd
            # cannot access PSUM)
            for nt in range(4):
                nc.vector.tensor_scalar(
                    stash[nt][:], accs3[nt][:], a128[:, 1:2], None,
                    mybir.AluOpType.mult,
                )
                nc.vector.tensor_copy(accs3[nt][:], seed_lo)
            for nt in range(4, RCH):
                nc.scalar.mul(stash[nt][:], accs3[nt][:], a128[:, 1:2])
                nc.scalar.copy(accs3[nt][:], seed_lo)
            for idx, (j, g, c) in enumerate(LO_PAIRS):
                strip = strips3.tile([P, 2, ROWS], cdt, name=f"rl_{j}",
                                     tag="strip3")
                nc.sync.dma_start(strip[:], s0t_v[:, j : j + 2, :])
                for u in range(2):
                    for nt in range(RCH):
                        nc.tensor.matmul(
                            accs3[nt][:],
                            strip[:, u, P * nt : P * (nt + 1)],
                            t_sb[:, j + u, :],
                            start=False,
                            stop=(idx == len(LO_PAIRS) - 1 and u == 1),
                        )
            for nt in range(RCH):
                lo = epi.tile([P, D], f32, name=f"elo_{nt}", tag="elo")
                # fused (acc * a0) max stash in one DVE pass; bias already in
                nc.vector.scalar_tensor_tensor(
                    lo[:], accs3[nt][:], a128[:, 0:1], stash[nt][:],
                    mybir.AluOpType.mult, mybir.AluOpType.max,
                )
                osb = epi.tile([P, D], f32, name=f"osb_{nt}", tag="osb")
                if nt % 2 == 0:
                    nc.scalar.activation(
                        osb[:], lo[:], mybir.ActivationFunctionType.Relu
                    )
                else:
                    nc.gpsimd.tensor_relu(osb[:], lo[:])
                row0 = P * nt
                store_eng = nc.sync if nt % 2 == 0 else nc.scalar
                store_eng.dma_start(out[row0 : row0 + P, :], osb[:])

    nc.compile()
    return nc


def _get_nc():
    if "nc" not in _CACHE:
        _CACHE["nc"] = _build_nc()
    return _CACHE["nc"]


def _shard_inputs(x, weights, alpha, bias, s0, s1, s2, s3):
    import ml_dtypes

    cnp = ml_dtypes.bfloat16

    def prep(a):  # transpose + cast, C-contiguous
        return np.ascontiguousarray(a.T).astype(cnp, copy=False)

    alpha = np.ascontiguousarray(alpha, dtype=np.float32)
    bias = np.ascontiguousarray(bias, dtype=np.float32)
    w_p = np.ascontiguousarray(weights).astype(cnp, copy=False)
    x_p = np.ascontiguousarray(x).astype(cnp, copy=False)  # natural layout
    in_maps = []
    for c in range(N_CORES):
        r0, r1 = ROWS * c, ROWS * (c + 1)
        # S = concat(s1, s3) rows; core c owns rows [r0, r1)
        if r1 <= K:
            s_rows = s1[r0:r1]
        elif r0 >= K:
            s_rows = s3[r0 - K : r1 - K]
        else:  # straddles the boundary (not the case for these shapes)
            s_rows = np.concatenate([s1[r0:], s3[: r1 - K]], axis=0)
        in_maps.append(
            {
                "x": x_p,
                "w": w_p,
                "alpha": alpha,
                "bias": bias,
                "st": prep(s_rows),
                "s0t": prep(s0[r0:r1]),
                "s2t": prep(s2[r0:r1]),
            }
        )
    return in_maps


def kernel(x, weights, alpha, bias, s0, s1, s2, s3, _trace=False):
    from concourse.bass_utils import run_bass_kernel_spmd

    nc = _get_nc()
    in_maps = _shard_inputs(
        np.asarray(x), np.asarray(weights), np.asarray(alpha), np.asarray(bias),
        np.asarray(s0), np.asarray(s1), np.asarray(s2), np.asarray(s3),
    )
    kwargs = {}
    if _trace:
        # warm-up execution: compile + collective init + allocator warm so the
        # traced run measures steady-state
        run_bass_kernel_spmd(nc, in_maps, core_ids=list(range(N_CORES)))
        kwargs = dict(trace=True, trace_cores=list(range(N_CORES)))
    r = run_bass_kernel_spmd(nc, in_maps, core_ids=list(range(N_CORES)), **kwargs)
    full = np.concatenate([res["out"] for res in r.results], axis=0)
    if _trace:
        return full, r
    return full


# revision 18
# speedup vs baseline: 1.0756x; 1.0756x over previous
"""SpGAT_Conv Trainium2 kernel: 8-core SPMD spectral GNN conv.

Math (reference):
    a = softmax(alpha)
    pre = x @ W                                   [N, D]
    out_low  = s0 @ (a0 * (s1 @ pre))             [N, D]
    out_high = s2 @ (a1 * (s3 @ pre))             [N, D]
    out = relu(max(out_low, out_high) + bias)

Re-association: t = S @ (x @ W) == (S @ x) @ W with S = concat(s1, s3).
Row-sharding t's rows across 8 cores makes the x@W work perfectly sharded
too (it rides on each core's own 1024 rows of u = S_c @ x) instead of being
replicated, cutting per-core PE work from 1280 to 1056 big matmuls:

    step 1: u_c^T = x^T S_c^T accumulated over n-chunks; stationary = x
            chunks (natural layout), moving = S_c^T strips.  Two sweeps of
            512 i-columns each (4 PSUM banks per sweep).
    step 2: t_c = u_c @ W via u^T slices stationary, W moving (32 matmuls);
            each sweep's 512 t rows are staged + AllGathered in two
            sub-collectives (4 total, order 0,1,3,2) while later compute
            runs.
    phase 3: out_c = relu(max(a0*s0_c@t1 + bias, a1*s2_c@t3 + bias)), high
             band then low band over ARRIVAL-ordered t chunks; PSUM is
             pre-seeded with bias/a so no separate bias add exists in the
             epilogue.

DMA discipline: DMA_DIRECT2D executes synchronously on the issuing
engine's queue AND all queues share one small DMA-completion semaphore
pool, so a blocked DMA (or a semaphore-reset rendezvous with one) stalls
unrelated queues.  Mitigations:
  - Sync queue: only matmul-pacing strip loads, t_in stores, half the out
    stores — never anything collective-gated.
  - Activation queue: bulk x/w loads (head), AllGather-gated t_sb loads
    (tail), half the out stores.
  - Batched 3D-AP transfers minimize DMA instruction count (≈92 total) so
    semaphore epochs recycle rarely.
  - Vector/GpSimd split all PSUM drain/seed/stash/epilogue work.

All big operands are host-cast to bf16 (full PE rate) with fp32 PSUM
accumulation; s-matrices are host-transposed so the contraction dim lands
on SBUF partitions with contiguous DMAs.  x needs no transpose in this
formulation.
"""

import os

import numpy as np

N_CORES = 8
N = 8192
K = 2048
NK = N - K          # 6144
D = 512
ROWS = N // N_CORES  # 1024 rows per core
P = 128
RCH = ROWS // P      # 8  (row chunks per core / output strips)
NCH = N // P         # 64 (contraction chunks over full N)
KCH = K // P         # 16 (low-band chunks; high band = NCH - KCH = 48)
DCH = D // P         # 4  (depth chunks)
NSUB = 4             # sub-AllGathers for t
SUBR = ROWS // NSUB  # 256 rows per rank per sub-AG
SB = 4               # n-chunks per sweep-strip DMA batch

DEBUG = os.environ.get("SPGAT_DEBUG", "0") == "1"

_CACHE = {}

# t-chunk arrival order: sub-AG g delivers, for every rank c, t rows
# [1024c + 256g, 1024c + 256(g+1)) = global chunks 8c + 2g + {0,1}.
# Gathers are triggered in order 0, 1, 3, 2 (sweep B stages its upper rows
# first), so consume in that order too.  Phase 3 iterates PAIRS (both u's
# of one (g, c)) so each pair is one strip DMA.
GORDER = [0, 1, 3, 2]
PAIRS = [(8 * c + 2 * g, g, c) for g in GORDER for c in range(N_CORES)]


def _build_nc():
    import concourse.mybir as mybir
    import concourse.tile as tile
    from concourse import bacc

    f32 = mybir.dt.float32
    bf16 = mybir.dt.bfloat16
    cdt = bf16

    nc = bacc.Bacc(
        "TRN2", target_bir_lowering=False, debug=False, num_devices=N_CORES
    )

    x = nc.dram_tensor("x", [N, D], cdt, kind="ExternalInput").ap()
    w = nc.dram_tensor("w", [D, D], cdt, kind="ExternalInput").ap()
    alpha = nc.dram_tensor("alpha", [2], f32, kind="ExternalInput").ap()
    bias = nc.dram_tensor("bias", [D], f32, kind="ExternalInput").ap()
    st = nc.dram_tensor("st", [N, ROWS], cdt, kind="ExternalInput").ap()
    s0t = nc.dram_tensor("s0t", [K, ROWS], cdt, kind="ExternalInput").ap()
    s2t = nc.dram_tensor("s2t", [NK, ROWS], cdt, kind="ExternalInput").ap()
    out = nc.dram_tensor("out", [ROWS, D], f32, kind="ExternalOutput").ap()
    if DEBUG:
        t_dump = nc.dram_tensor("t_dump", [N, D], cdt, kind="ExternalOutput").ap()

    groups = [list(range(N_CORES))]

    with tile.TileContext(nc) as tc:
        with (
            tc.tile_pool(name="const", bufs=1) as const,
            tc.tile_pool(name="bigA", bufs=1) as bigA,
            tc.tile_pool(name="bigB", bufs=1) as bigB,
            tc.tile_pool(name="strips1", bufs=3) as strips1,
            tc.tile_pool(name="strips3", bufs=3) as strips3,
            tc.tile_pool(name="stage", bufs=4) as stage,
            tc.tile_pool(name="epi", bufs=2) as epi,
            tc.tile_pool(name="stash", bufs=1) as stashp,
            tc.tile_pool(name="ps", bufs=8, space="PSUM") as ps,
            tc.tile_pool(name="dram", bufs=1, space="DRAM") as dram,
        ):
            # ---- collective warm-up: absorb first-collective init.  No
            # input deps so the trigger is the very first gpsimd instruction.
            warm_in = dram.tile([8, 8], f32, name="warm_in")
            warm_out = dram.tile([64, 8], f32, name="warm_out", addr_space="Shared")
            nc.gpsimd.collective_compute(
                "AllGather",
                mybir.AluOpType.bypass,
                replica_groups=groups,
                ins=[warm_in.opt()],
                outs=[warm_out.opt()],
            )

            # ---- PSUM allocation order fixes the 8-bank rotation; all
            # tiles are one 2KB bank.
            ps_a = ps.tile([P, 2], f32, name="ps_a", tag="acc")
            ps_b0 = ps.tile([P, D], f32, name="ps_b0", tag="acc")
            ps_b1 = ps.tile([P, D], f32, name="ps_b1", tag="acc")
            accsA = [ps.tile([P, D], f32, name=f"uA_{dc}", tag="acc")
                     for dc in range(DCH)]
            accsB = [ps.tile([P, D], f32, name=f"uB_{dc}", tag="acc")
                     for dc in range(DCH)]

            # ---- input DMAs: alpha/bias tiny on sync; x/w bulk on the
            # scalar (Activation) HWDGE queue so the sync queue only ever
            # carries matmul-pacing strip loads.
            asb = const.tile([1, 2], f32, name="asb")
            nc.sync.dma_start(asb[:], alpha[None, :])
            bsb = const.tile([1, D], f32, name="bsb")
            nc.sync.dma_start(bsb[:], bias[None, :])

            x_sb = bigA.tile([P, NCH, D], cdt, name="x_sb", tag="bigA")
            x_v = x.rearrange("(c p) d -> p c d", p=P)
            # block sizes pace delivery against sweep-A consumption: small
            # leading blocks so the first matmuls start fast, growing blocks
            # that always complete before the PE reaches them
            for c0, c1 in ((0, 1), (1, 2), (2, 4), (4, 8), (8, 12), (12, 16),
                           (16, 24), (24, 32), (32, 40), (40, 48), (48, 56),
                           (56, 64)):
                nc.scalar.dma_start(x_sb[:, c0:c1, :], x_v[:, c0:c1, :])
            w_sb = const.tile([P, DCH, D], cdt, name="w_sb")
            nc.scalar.dma_start(w_sb[:], w.rearrange("(c p) d -> p c d", p=P))

            # ---- steps 1+2 infrastructure
            t_in = dram.tile([ROWS, D], cdt, name="t_in")
            t_outs = [
                dram.tile([SUBR * N_CORES, D], cdt, name=f"t_out{g}",
                          addr_space="Shared")
                for g in range(NSUB)
            ]

            def t_subag(g):
                nc.gpsimd.collective_compute(
                    "AllGather",
                    mybir.AluOpType.bypass,
                    replica_groups=groups,
                    ins=[t_in[SUBR * g : SUBR * (g + 1), :].opt()],
                    outs=[t_outs[g].opt()],
                )

            ut_sb = [
                const.tile([P, DCH, D], cdt, name=f"ut{sw}") for sw in range(2)
            ]

            st_v = st.rearrange("(c p) i -> p c i", p=P)

            def sweep1(sw, accs, b0, b1, split_first=False):
                """step-1 matmuls for strip batches [b0, b1) of sweep sw;
                each batch is SB n-chunks in one DMA."""
                col0 = D * sw
                for bk in range(b0, b1):
                    j0 = SB * bk
                    strip = strips1.tile([P, SB, D], cdt, name=f"s{sw}_{bk}",
                                         tag="strip")
                    if split_first and bk == b0:
                        for j2 in range(SB):
                            nc.sync.dma_start(
                                strip[:, j2 : j2 + 1, :],
                                st_v[:, j0 + j2 : j0 + j2 + 1,
                                     col0 : col0 + D],
                            )
                    else:
                        nc.sync.dma_start(
                            strip[:], st_v[:, j0 : j0 + SB, col0 : col0 + D]
                        )
                    for j2 in range(SB):
                        j = j0 + j2
                        for dc in range(DCH):
                            nc.tensor.matmul(
                                accs[dc][:],
                                x_sb[:, j, P * dc : P * (dc + 1)],
                                strip[:, j2, :],
                                start=(j == 0),
                                stop=(j == NCH - 1),
                            )

            def drain_u(sw, accs):
                for dc in range(DCH):
                    nc.vector.tensor_copy(ut_sb[sw][:, dc, :], accs[dc][:])

            def step2(sw, tps=None):
                """t rows [512sw, 512sw+512): 4 i-blocks of 128 rows; stage
                + trigger this sweep's two sub-AGs (B stages upper first)."""
                ib_order = [0, 1, 2, 3] if sw == 0 else [2, 3, 0, 1]
                ag_map = {1: 0, 3: 1} if sw == 0 else {1: 3, 3: 2}
                for i, ib in enumerate(ib_order):
                    if tps is None:
                        tp = ps.tile([P, D], f32, name=f"t_ps_{sw}_{ib}",
                                     tag="acc")
                    else:
                        tp = tps[i]
                    for dc in range(DCH):
                        nc.tensor.matmul(
                            tp[:],
                            ut_sb[sw][:, dc, P * ib : P * (ib + 1)],
                            w_sb[:, dc, :],
                            start=(dc == 0),
                            stop=(dc == DCH - 1),
                        )
                    tst = stage.tile([P, D], cdt, name=f"t_st_{sw}_{ib}",
                                     tag="st")
                    nc.vector.tensor_copy(tst[:], tp[:])
                    row0 = D * sw + P * ib
                    nc.sync.dma_start(t_in[row0 : row0 + P, :], tst[:])
                    if i in ag_map:
                        t_subag(ag_map[i])

            NB = NCH // SB  # 16 strip batches per sweep

            # ---- PE stream head: first strip batch goes before the softmax
            # setup matmuls so the PE starts as soon as chunk 0 lands.
            sweep1(0, accsA, 0, 1, split_first=True)

            # softmax(alpha) chain (vector) + broadcasts via tiny matmuls
            amax = const.tile([1, 1], f32, name="amax")
            nc.vector.tensor_tensor(
                amax[:], asb[:, 0:1], asb[:, 1:2], mybir.AluOpType.max
            )
            ash = const.tile([1, 2], f32, name="ash")
            nc.vector.tensor_scalar(
                ash[:], asb[:], amax[:, 0:1], None, mybir.AluOpType.subtract
            )
            aexp = const.tile([1, 2], f32, name="aexp")
            nc.scalar.activation(aexp[:], ash[:], mybir.ActivationFunctionType.Exp)
            asum = const.tile([1, 1], f32, name="asum")
            nc.vector.tensor_tensor(
                asum[:], aexp[:, 0:1], aexp[:, 1:2], mybir.AluOpType.add
            )
            arec = const.tile([1, 1], f32, name="arec")
            nc.vector.reciprocal(arec[:], asum[:])
            afin = const.tile([1, 2], f32, name="afin")
            nc.vector.tensor_scalar(
                afin[:], aexp[:], arec[:, 0:1], None, mybir.AluOpType.mult
            )
            ainv = const.tile([1, 2], f32, name="ainv")
            nc.vector.reciprocal(ainv[:], afin[:])
            srow0 = const.tile([1, D], f32, name="srow0")
            nc.vector.tensor_scalar(
                srow0[:], bsb[:], ainv[:, 0:1], None, mybir.AluOpType.mult
            )
            srow1 = const.tile([1, D], f32, name="srow1")
            nc.vector.tensor_scalar(
                srow1[:], bsb[:], ainv[:, 1:2], None, mybir.AluOpType.mult
            )
            ones = const.tile([1, P], f32, name="ones")
            nc.vector.memset(ones[:], 1.0)
            nc.tensor.matmul(ps_a[:], ones[:], afin[:], start=True, stop=True)
            a128 = const.tile([P, 2], f32, name="a128")
            nc.vector.tensor_copy(a128[:], ps_a[:])
            seeds = const.tile([P, 2 * D], f32, name="seeds")
            nc.tensor.matmul(ps_b0[:], ones[:], srow0[:], start=True, stop=True)
            nc.vector.tensor_copy(seeds[:, 0:D], ps_b0[:])
            nc.tensor.matmul(ps_b1[:], ones[:], srow1[:], start=True, stop=True)
            nc.vector.tensor_copy(seeds[:, D : 2 * D], ps_b1[:])
            seed_lo = seeds[:, 0:D]
            seed_hi = seeds[:, D : 2 * D]

            # ---- rest of sweep A, then pipelined B/step2
            sweep1(0, accsA, 1, NB)
            drain_u(0, accsA)
            # head of sweep B hides sweep A's drain latency before step2A
            sweep1(1, accsB, 0, 2)
            step2(0)
            sweep1(1, accsB, 2, NB)
            drain_u(1, accsB)

            # PSUM rotation: allocate step2B's tiles, then phase-3
            # accumulators — accs3[0..3] land on step2A's banks (free
            # early), accs3[4..7] on step2B's.  Seed 0..3 before step2B's
            # instructions so only 4..7 wait on its staging.
            tps1 = [ps.tile([P, D], f32, name=f"t_ps_1_{i}", tag="acc")
                    for i in range(4)]
            accs3 = [
                ps.tile([P, D], f32, name=f"acc3_{nt}", tag="acc")
                for nt in range(RCH)
            ]
            for nt in range(4):
                nc.vector.tensor_copy(accs3[nt][:], seed_hi)
            step2(1, tps1)
            for nt in range(4, RCH):
                nc.scalar.copy(accs3[nt][:], seed_hi)

            # ---- phase 3: t gather consumption + band matmuls.  One t_sb
            # load per (sub-AG, u): 8 gather DMAs total on the scalar queue.
            # t_sb loads ride the otherwise-idle gpsimd queue: their
            # AllGather-gated waits then cannot rendezvous-block the two
            # HWDGE queues through the shared semaphore pool.
            t_sb = bigB.tile([P, NCH, D], cdt, name="t_sb", tag="bigB")
            t_sb_r = t_sb[:].rearrange("p (c r) d -> p r c d", r=8)
            for g in GORDER:
                for u in range(2):
                    # chunks j = 8c + 2g + u for c in 0..7
                    nc.gpsimd.dma_start(
                        t_sb_r[:, 2 * g + u, :, :],
                        t_outs[g].rearrange("(c q p) d -> p c q d", p=P, q=2)[
                            :, :, u, :
                        ],
                    )
            if DEBUG:
                for j in range(NCH):
                    nc.sync.dma_start(
                        t_dump[P * j : P * (j + 1), :], t_sb[:, j, :]
                    )

            HI_PAIRS = [e for e in PAIRS if e[0] >= KCH]
            LO_PAIRS = [e for e in PAIRS if e[0] < KCH]
            stash = [
                stashp.tile([P, D], f32, name=f"hst_{nt}", tag=f"hst{nt}")
                for nt in range(RCH)
            ]
            s2t_v = s2t.rearrange("(c p) i -> p c i", p=P)
            s0t_v = s0t.rearrange("(c p) i -> p c i", p=P)
            for idx, (j, g, c) in enumerate(HI_PAIRS):
                jj = j - KCH
                strip = strips3.tile([P, 2, ROWS], cdt, name=f"rh_{j}",
                                     tag="strip3")
                nc.sync.dma_start(strip[:], s2t_v[:, jj : jj + 2, :])
                for u in range(2):
                    for nt in range(RCH):
                        nc.tensor.matmul(
                            accs3[nt][:],
                            strip[:, u, P * nt : P * (nt + 1)],
                            t_sb[:, j + u, :],
                            start=False,
                            stop=(idx == len(HI_PAIRS) - 1 and u == 1),
                        )
            # stash = a1*acc = a1*hi + bias, then re-seed for the low band
            # right behind the stash read; split vector/scalar (gpsimd
            # cannot access PSUM)
            for nt in range(4):
                nc.vector.tensor_scalar(
                    stash[nt][:], accs3[nt][:], a128[:, 1:2], None,
                    mybir.AluOpType.mult,
                )
                nc.vector.tensor_copy(accs3[nt][:], seed_lo)
            for nt in range(4, RCH):
                nc.scalar.mul(stash[nt][:], accs3[nt][:], a128[:, 1:2])
                nc.scalar.copy(accs3[nt][:], seed_lo)
            # relu the stash in place (off the critical path, during the low
            # band): relu(max(u,v)) == max(relu(u), relu(v)) lets the final
            # per-strip chain be one ACT op + one DVE max
            for nt in range(RCH):
                nc.scalar.activation(
                    stash[nt][:], stash[nt][:],
                    mybir.ActivationFunctionType.Relu,
                )
            for idx, (j, g, c) in enumerate(LO_PAIRS):
                strip = strips3.tile([P, 2, ROWS], cdt, name=f"rl_{j}",
                                     tag="strip3")
                nc.sync.dma_start(strip[:], s0t_v[:, j : j + 2, :])
                for u in range(2):
                    for nt in range(RCH):
                        nc.tensor.matmul(
                            accs3[nt][:],
                            strip[:, u, P * nt : P * (nt + 1)],
                            t_sb[:, j + u, :],
                            start=False,
                            stop=(idx == len(LO_PAIRS) - 1 and u == 1),
                        )
            for nt in range(RCH):
                lo = epi.tile([P, D], f32, name=f"elo_{nt}", tag="elo")
                # relu(a0*acc) straight out of PSUM (bias already seeded in)
                nc.scalar.activation(
                    lo[:], accs3[nt][:], mybir.ActivationFunctionType.Relu,
                    scale=a128[:, 0:1],
                )
                osb = epi.tile([P, D], f32, name=f"osb_{nt}", tag="osb")
                nc.vector.tensor_tensor(
                    osb[:], lo[:], stash[nt][:], mybir.AluOpType.max
                )
                row0 = P * nt
                nc.sync.dma_start(out[row0 : row0 + P, :], osb[:])

    nc.compile()
    return nc


def _get_nc():
    if "nc" not in _CACHE:
        _CACHE["nc"] = _build_nc()
    return _CACHE["nc"]


def _shard_inputs(x, weights, alpha, bias, s0, s1, s2, s3):
    import ml_dtypes

    cnp = ml_dtypes.bfloat16

    def prep(a):  # transpose + cast, C-contiguous
        return np.ascontiguousarray(a.T).astype(cnp, copy=False)

    alpha = np.ascontiguousarray(alpha, dtype=np.float32)
    bias = np.ascontiguousarray(bias, dtype=np.float32)
    w_p = np.ascontiguousarray(weights).astype(cnp, copy=False)
    x_p = np.ascontiguousarray(x).astype(cnp, copy=False)  # natural layout
    in_maps = []
    for c in range(N_CORES):
        r0, r1 = ROWS * c, ROWS * (c + 1)
        # S = concat(s1, s3) rows; core c owns rows [r0, r1)
        if r1 <= K:
            s_rows = s1[r0:r1]
        elif r0 >= K:
            s_rows = s3[r0 - K : r1 - K]
        else:  # straddles the boundary (not the case for these shapes)
            s_rows = np.concatenate([s1[r0:], s3[: r1 - K]], axis=0)
        in_maps.append(
            {
                "x": x_p,
                "w": w_p,
                "alpha": alpha,
                "bias": bias,
                "st": prep(s_rows),
                "s0t": prep(s0[r0:r1]),
                "s2t": prep(s2[r0:r1]),
            }
        )
    return in_maps


def kernel(x, weights, alpha, bias, s0, s1, s2, s3, _trace=False):
    from concourse.bass_utils import run_bass_kernel_spmd

    nc = _get_nc()
    in_maps = _shard_inputs(
        np.asarray(x), np.asarray(weights), np.asarray(alpha), np.asarray(bias),
        np.asarray(s0), np.asarray(s1), np.asarray(s2), np.asarray(s3),
    )
    kwargs = {}
    if _trace:
        # warm-up execution: compile + collective init + allocator warm so the
        # traced run measures steady-state
        run_bass_kernel_spmd(nc, in_maps, core_ids=list(range(N_CORES)))
        kwargs = dict(trace=True, trace_cores=list(range(N_CORES)))
    r = run_bass_kernel_spmd(nc, in_maps, core_ids=list(range(N_CORES)), **kwargs)
    full = np.concatenate([res["out"] for res in r.results], axis=0)
    if _trace:
        return full, r
    return full


# revision 19
# speedup vs baseline: 1.1234x; 1.0444x over previous
"""SpGAT_Conv Trainium2 kernel: 8-core SPMD spectral GNN conv.

Math (reference):
    a = softmax(alpha)
    pre = x @ W                                   [N, D]
    out_low  = s0 @ (a0 * (s1 @ pre))             [N, D]
    out_high = s2 @ (a1 * (s3 @ pre))             [N, D]
    out = relu(max(out_low, out_high) + bias)

Re-association: t = S @ (x @ W) == (S @ x) @ W with S = concat(s1, s3).
Row-sharding t's rows across 8 cores makes the x@W work perfectly sharded
too (it rides on each core's own 1024 rows of u = S_c @ x) instead of being
replicated, cutting per-core PE work from 1280 to 1056 big matmuls:

    step 1: u_c^T = x^T S_c^T accumulated over n-chunks; stationary = x
            chunks (natural layout), moving = S_c^T strips.  Two sweeps of
            512 i-columns each (4 PSUM banks per sweep).
    step 2: t_c = u_c @ W via u^T slices stationary, W moving (32 matmuls);
            each sweep's 512 t rows are staged + AllGathered in two
            sub-collectives (4 total, order 0,1,3,2) while later compute
            runs.
    phase 3: out_c = relu(max(a0*s0_c@t1 + bias, a1*s2_c@t3 + bias)), high
             band then low band over ARRIVAL-ordered t chunks; PSUM is
             pre-seeded with bias/a so no separate bias add exists in the
             epilogue.

DMA discipline: DMA_DIRECT2D executes synchronously on the issuing
engine's queue AND all queues share one small DMA-completion semaphore
pool, so a blocked DMA (or a semaphore-reset rendezvous with one) stalls
unrelated queues.  Mitigations:
  - Sync queue: only matmul-pacing strip loads, t_in stores, half the out
    stores — never anything collective-gated.
  - Activation queue: bulk x/w loads (head), AllGather-gated t_sb loads
    (tail), half the out stores.
  - Batched 3D-AP transfers minimize DMA instruction count (≈92 total) so
    semaphore epochs recycle rarely.
  - Vector/GpSimd split all PSUM drain/seed/stash/epilogue work.

All big operands are host-cast to bf16 (full PE rate) with fp32 PSUM
accumulation; s-matrices are host-transposed so the contraction dim lands
on SBUF partitions with contiguous DMAs.  x needs no transpose in this
formulation.
"""

import os

import numpy as np

N_CORES = 8
N = 8192
K = 2048
NK = N - K          # 6144
D = 512
ROWS = N // N_CORES  # 1024 rows per core
P = 128
RCH = ROWS // P      # 8  (row chunks per core / output strips)
NCH = N // P         # 64 (contraction chunks over full N)
KCH = K // P         # 16 (low-band chunks; high band = NCH - KCH = 48)
DCH = D // P         # 4  (depth chunks)
NSUB = 4             # sub-AllGathers for t
SUBR = ROWS // NSUB  # 256 rows per rank per sub-AG
SB = 4               # n-chunks per sweep-strip DMA batch

DEBUG = os.environ.get("SPGAT_DEBUG", "0") == "1"

_CACHE = {}

# t-chunk arrival order: sub-AG g delivers, for every rank c, t rows
# [1024c + 256g, 1024c + 256(g+1)) = global chunks 8c + 2g + {0,1}.
# Gathers are triggered in order 0, 1, 3, 2 (sweep B stages its upper rows
# first), so consume in that order too.  Phase 3 iterates PAIRS (both u's
# of one (g, c)) so each pair is one strip DMA.
GORDER = [0, 1, 3, 2]
PAIRS = [(8 * c + 2 * g, g, c) for g in GORDER for c in range(N_CORES)]


def _build_nc():
    import concourse.mybir as mybir
    import concourse.tile as tile
    from concourse import bacc

    f32 = mybir.dt.float32
    bf16 = mybir.dt.bfloat16
    cdt = bf16

    nc = bacc.Bacc(
        "TRN2", target_bir_lowering=False, debug=False, num_devices=N_CORES
    )

    # all big operands host-packed so every DMA reads 2KB+ contiguous
    # per-partition lines (max descriptor efficiency)
    xp = nc.dram_tensor("xp", [NCH // 2, P, 2 * D], cdt,
                        kind="ExternalInput").ap()
    w = nc.dram_tensor("w", [D, D], cdt, kind="ExternalInput").ap()
    alpha = nc.dram_tensor("alpha", [2], f32, kind="ExternalInput").ap()
    bias = nc.dram_tensor("bias", [D], f32, kind="ExternalInput").ap()
    stp = nc.dram_tensor("stp", [2, NCH // SB, P, SB * D], cdt,
                         kind="ExternalInput").ap()
    s0p = nc.dram_tensor("s0p", [KCH // 2, P, 2 * ROWS], cdt,
                         kind="ExternalInput").ap()
    s2p = nc.dram_tensor("s2p", [(NCH - KCH) // 2, P, 2 * ROWS], cdt,
                         kind="ExternalInput").ap()
    out = nc.dram_tensor("out", [ROWS, D], f32, kind="ExternalOutput").ap()
    if DEBUG:
        t_dump = nc.dram_tensor("t_dump", [N, D], cdt, kind="ExternalOutput").ap()

    groups = [list(range(N_CORES))]

    with tile.TileContext(nc) as tc:
        with (
            tc.tile_pool(name="const", bufs=1) as const,
            tc.tile_pool(name="bigA", bufs=1) as bigA,
            tc.tile_pool(name="bigB", bufs=1) as bigB,
            tc.tile_pool(name="strips1", bufs=3) as strips1,
            tc.tile_pool(name="strips3", bufs=3) as strips3,
            tc.tile_pool(name="stage", bufs=4) as stage,
            tc.tile_pool(name="epi", bufs=2) as epi,
            tc.tile_pool(name="stash", bufs=1) as stashp,
            tc.tile_pool(name="ps", bufs=8, space="PSUM") as ps,
            tc.tile_pool(name="dram", bufs=1, space="DRAM") as dram,
        ):
            # ---- collective warm-up: absorb first-collective init.  No
            # input deps so the trigger is the very first gpsimd instruction.
            warm_in = dram.tile([8, 8], f32, name="warm_in")
            warm_out = dram.tile([64, 8], f32, name="warm_out", addr_space="Shared")
            nc.gpsimd.collective_compute(
                "AllGather",
                mybir.AluOpType.bypass,
                replica_groups=groups,
                ins=[warm_in.opt()],
                outs=[warm_out.opt()],
            )

            # ---- PSUM allocation order fixes the 8-bank rotation; all
            # tiles are one 2KB bank.
            ps_a = ps.tile([P, 2], f32, name="ps_a", tag="acc")
            ps_b0 = ps.tile([P, D], f32, name="ps_b0", tag="acc")
            ps_b1 = ps.tile([P, D], f32, name="ps_b1", tag="acc")
            accsA = [ps.tile([P, D], f32, name=f"uA_{dc}", tag="acc")
                     for dc in range(DCH)]
            accsB = [ps.tile([P, D], f32, name=f"uB_{dc}", tag="acc")
                     for dc in range(DCH)]

            # ---- input DMAs: alpha/bias tiny on sync; x/w bulk on the
            # scalar (Activation) HWDGE queue so the sync queue only ever
            # carries matmul-pacing strip loads.
            asb = const.tile([1, 2], f32, name="asb")
            nc.sync.dma_start(asb[:], alpha[None, :])
            bsb = const.tile([1, D], f32, name="bsb")
            nc.sync.dma_start(bsb[:], bias[None, :])

            x_sb = bigA.tile([P, NCH, D], cdt, name="x_sb", tag="bigA")
            # block sizes (in 2-chunk units) pace delivery against sweep-A
            # consumption: small leading blocks so the first matmuls start
            # fast, growing blocks that always land before the PE needs them
            for b0, b1 in ((0, 1), (1, 2), (2, 4), (4, 6), (6, 8), (8, 12),
                           (12, 16), (16, 24), (24, 32)):
                nc.scalar.dma_start(
                    x_sb[:, 2 * b0 : 2 * b1, :].rearrange("p c d -> p (c d)"),
                    xp[b0:b1].rearrange("b p a -> p b a"),
                )
            w_sb = const.tile([P, DCH, D], cdt, name="w_sb")
            nc.scalar.dma_start(w_sb[:], w.rearrange("(c p) d -> p c d", p=P))

            # ---- steps 1+2 infrastructure
            t_in = dram.tile([ROWS, D], cdt, name="t_in")
            t_outs = [
                dram.tile([SUBR * N_CORES, D], cdt, name=f"t_out{g}",
                          addr_space="Shared")
                for g in range(NSUB)
            ]

            def t_subag(g):
                nc.gpsimd.collective_compute(
                    "AllGather",
                    mybir.AluOpType.bypass,
                    replica_groups=groups,
                    ins=[t_in[SUBR * g : SUBR * (g + 1), :].opt()],
                    outs=[t_outs[g].opt()],
                )

            ut_sb = [
                const.tile([P, DCH, D], cdt, name=f"ut{sw}") for sw in range(2)
            ]

            def sweep1(sw, accs, b0, b1, split_first=False):
                """step-1 matmuls for strip batches [b0, b1) of sweep sw;
                each batch is SB n-chunks in one contiguous-packed DMA."""
                for bk in range(b0, b1):
                    j0 = SB * bk
                    strip = strips1.tile([P, SB, D], cdt, name=f"s{sw}_{bk}",
                                         tag="strip")
                    if split_first and bk == b0:
                        for j2 in range(SB):
                            nc.sync.dma_start(
                                strip[:, j2, :],
                                stp[sw, bk][:, D * j2 : D * (j2 + 1)],
                            )
                    else:
                        nc.sync.dma_start(
                            strip[:].rearrange("p a b -> p (a b)"),
                            stp[sw, bk],
                        )
                    for j2 in range(SB):
                        j = j0 + j2
                        for dc in range(DCH):
                            nc.tensor.matmul(
                                accs[dc][:],
                                x_sb[:, j, P * dc : P * (dc + 1)],
                                strip[:, j2, :],
                                start=(j == 0),
                                stop=(j == NCH - 1),
                            )

            def drain_u(sw, accs):
                for dc in range(DCH):
                    nc.vector.tensor_copy(ut_sb[sw][:, dc, :], accs[dc][:])

            def step2(sw, tps=None):
                """t rows [512sw, 512sw+512): 4 i-blocks of 128 rows; stage
                + trigger this sweep's two sub-AGs (B stages upper first)."""
                ib_order = [0, 1, 2, 3] if sw == 0 else [2, 3, 0, 1]
                ag_map = {1: 0, 3: 1} if sw == 0 else {1: 3, 3: 2}
                for i, ib in enumerate(ib_order):
                    if tps is None:
                        tp = ps.tile([P, D], f32, name=f"t_ps_{sw}_{ib}",
                                     tag="acc")
                    else:
                        tp = tps[i]
                    for dc in range(DCH):
                        nc.tensor.matmul(
                            tp[:],
                            ut_sb[sw][:, dc, P * ib : P * (ib + 1)],
                            w_sb[:, dc, :],
                            start=(dc == 0),
                            stop=(dc == DCH - 1),
                        )
                    tst = stage.tile([P, D], cdt, name=f"t_st_{sw}_{ib}",
                                     tag="st")
                    nc.vector.tensor_copy(tst[:], tp[:])
                    row0 = D * sw + P * ib
                    nc.sync.dma_start(t_in[row0 : row0 + P, :], tst[:])
                    if i in ag_map:
                        t_subag(ag_map[i])

            NB = NCH // SB  # 16 strip batches per sweep

            # ---- PE stream head: first strip batch goes before the softmax
            # setup matmuls so the PE starts as soon as chunk 0 lands.
            sweep1(0, accsA, 0, 1, split_first=True)

            # softmax(alpha) chain (vector) + broadcasts via tiny matmuls
            amax = const.tile([1, 1], f32, name="amax")
            nc.vector.tensor_tensor(
                amax[:], asb[:, 0:1], asb[:, 1:2], mybir.AluOpType.max
            )
            ash = const.tile([1, 2], f32, name="ash")
            nc.vector.tensor_scalar(
                ash[:], asb[:], amax[:, 0:1], None, mybir.AluOpType.subtract
            )
            aexp = const.tile([1, 2], f32, name="aexp")
            nc.scalar.activation(aexp[:], ash[:], mybir.ActivationFunctionType.Exp)
            asum = const.tile([1, 1], f32, name="asum")
            nc.vector.tensor_tensor(
                asum[:], aexp[:, 0:1], aexp[:, 1:2], mybir.AluOpType.add
            )
            arec = const.tile([1, 1], f32, name="arec")
            nc.vector.reciprocal(arec[:], asum[:])
            afin = const.tile([1, 2], f32, name="afin")
            nc.vector.tensor_scalar(
                afin[:], aexp[:], arec[:, 0:1], None, mybir.AluOpType.mult
            )
            ainv = const.tile([1, 2], f32, name="ainv")
            nc.vector.reciprocal(ainv[:], afin[:])
            srow0 = const.tile([1, D], f32, name="srow0")
            nc.vector.tensor_scalar(
                srow0[:], bsb[:], ainv[:, 0:1], None, mybir.AluOpType.mult
            )
            srow1 = const.tile([1, D], f32, name="srow1")
            nc.vector.tensor_scalar(
                srow1[:], bsb[:], ainv[:, 1:2], None, mybir.AluOpType.mult
            )
            ones = const.tile([1, P], f32, name="ones")
            nc.vector.memset(ones[:], 1.0)
            nc.tensor.matmul(ps_a[:], ones[:], afin[:], start=True, stop=True)
            a128 = const.tile([P, 2], f32, name="a128")
            nc.vector.tensor_copy(a128[:], ps_a[:])
            seeds = const.tile([P, 2 * D], f32, name="seeds")
            nc.tensor.matmul(ps_b0[:], ones[:], srow0[:], start=True, stop=True)
            nc.vector.tensor_copy(seeds[:, 0:D], ps_b0[:])
            nc.tensor.matmul(ps_b1[:], ones[:], srow1[:], start=True, stop=True)
            nc.vector.tensor_copy(seeds[:, D : 2 * D], ps_b1[:])
            seed_lo = seeds[:, 0:D]
            seed_hi = seeds[:, D : 2 * D]

            # ---- rest of sweep A, then pipelined B/step2
            sweep1(0, accsA, 1, NB)
            drain_u(0, accsA)
            # head of sweep B hides sweep A's drain latency before step2A
            sweep1(1, accsB, 0, 2)
            step2(0)
            sweep1(1, accsB, 2, NB)
            drain_u(1, accsB)

            # PSUM rotation: allocate step2B's tiles, then phase-3
            # accumulators — accs3[0..3] land on step2A's banks (free
            # early), accs3[4..7] on step2B's.  Seed 0..3 before step2B's
            # instructions so only 4..7 wait on its staging.
            tps1 = [ps.tile([P, D], f32, name=f"t_ps_1_{i}", tag="acc")
                    for i in range(4)]
            accs3 = [
                ps.tile([P, D], f32, name=f"acc3_{nt}", tag="acc")
                for nt in range(RCH)
            ]
            for nt in range(4):
                nc.vector.tensor_copy(accs3[nt][:], seed_hi)
            step2(1, tps1)
            for nt in range(4, RCH):
                nc.scalar.copy(accs3[nt][:], seed_hi)

            # ---- phase 3: t gather consumption + band matmuls.  One t_sb
            # load per (sub-AG, u): 8 gather DMAs total on the scalar queue.
            # t_sb loads ride the otherwise-idle gpsimd queue: their
            # AllGather-gated waits then cannot rendezvous-block the two
            # HWDGE queues through the shared semaphore pool.
            t_sb = bigB.tile([P, NCH, D], cdt, name="t_sb", tag="bigB")
            t_sb_r = t_sb[:].rearrange("p (c r) d -> p r c d", r=8)
            for g in GORDER:
                for u in range(2):
                    # chunks j = 8c + 2g + u for c in 0..7
                    nc.gpsimd.dma_start(
                        t_sb_r[:, 2 * g + u, :, :],
                        t_outs[g].rearrange("(c q p) d -> p c q d", p=P, q=2)[
                            :, :, u, :
                        ],
                    )
            if DEBUG:
                for j in range(NCH):
                    nc.sync.dma_start(
                        t_dump[P * j : P * (j + 1), :], t_sb[:, j, :]
                    )

            HI_PAIRS = [e for e in PAIRS if e[0] >= KCH]
            LO_PAIRS = [e for e in PAIRS if e[0] < KCH]
            stash = [
                stashp.tile([P, D], f32, name=f"hst_{nt}", tag=f"hst{nt}")
                for nt in range(RCH)
            ]
            for idx, (j, g, c) in enumerate(HI_PAIRS):
                jj = j - KCH
                strip = strips3.tile([P, 2, ROWS], cdt, name=f"rh_{j}",
                                     tag="strip3")
                nc.sync.dma_start(
                    strip[:].rearrange("p a b -> p (a b)"), s2p[jj // 2]
                )
                for u in range(2):
                    for nt in range(RCH):
                        nc.tensor.matmul(
                            accs3[nt][:],
                            strip[:, u, P * nt : P * (nt + 1)],
                            t_sb[:, j + u, :],
                            start=False,
                            stop=(idx == len(HI_PAIRS) - 1 and u == 1),
                        )
            # stash = a1*acc = a1*hi + bias, then re-seed for the low band
            # right behind the stash read; split vector/scalar (gpsimd
            # cannot access PSUM)
            for nt in range(4):
                nc.vector.tensor_scalar(
                    stash[nt][:], accs3[nt][:], a128[:, 1:2], None,
                    mybir.AluOpType.mult,
                )
                nc.vector.tensor_copy(accs3[nt][:], seed_lo)
            for nt in range(4, RCH):
                nc.scalar.mul(stash[nt][:], accs3[nt][:], a128[:, 1:2])
                nc.scalar.copy(accs3[nt][:], seed_lo)
            # relu the stash in place (off the critical path, during the low
            # band): relu(max(u,v)) == max(relu(u), relu(v)) lets the final
            # per-strip chain be one ACT op + one DVE max
            for nt in range(RCH):
                nc.scalar.activation(
                    stash[nt][:], stash[nt][:],
                    mybir.ActivationFunctionType.Relu,
                )
            for idx, (j, g, c) in enumerate(LO_PAIRS):
                strip = strips3.tile([P, 2, ROWS], cdt, name=f"rl_{j}",
                                     tag="strip3")
                nc.sync.dma_start(
                    strip[:].rearrange("p a b -> p (a b)"), s0p[j // 2]
                )
                for u in range(2):
                    for nt in range(RCH):
                        nc.tensor.matmul(
                            accs3[nt][:],
                            strip[:, u, P * nt : P * (nt + 1)],
                            t_sb[:, j + u, :],
                            start=False,
                            stop=(idx == len(LO_PAIRS) - 1 and u == 1),
                        )
            for nt in range(RCH):
                lo = epi.tile([P, D], f32, name=f"elo_{nt}", tag="elo")
                # relu(a0*acc) straight out of PSUM (bias already seeded in)
                nc.scalar.activation(
                    lo[:], accs3[nt][:], mybir.ActivationFunctionType.Relu,
                    scale=a128[:, 0:1],
                )
                osb = epi.tile([P, D], f32, name=f"osb_{nt}", tag="osb")
                nc.vector.tensor_tensor(
                    osb[:], lo[:], stash[nt][:], mybir.AluOpType.max
                )
                row0 = P * nt
                nc.sync.dma_start(out[row0 : row0 + P, :], osb[:])

    nc.compile()
    return nc


def _get_nc():
    if "nc" not in _CACHE:
        _CACHE["nc"] = _build_nc()
    return _CACHE["nc"]


def _shard_inputs(x, weights, alpha, bias, s0, s1, s2, s3):
    import ml_dtypes

    cnp = ml_dtypes.bfloat16

    def prep(a):  # transpose + cast, C-contiguous
        return np.ascontiguousarray(a.T).astype(cnp, copy=False)

    alpha = np.ascontiguousarray(alpha, dtype=np.float32)
    bias = np.ascontiguousarray(bias, dtype=np.float32)
    w_p = np.ascontiguousarray(weights).astype(cnp, copy=False)
    # xp[b, p, (j2 d)] = x[128*(2b+j2)+p, d]: 2KB contiguous per partition
    x_p = np.ascontiguousarray(
        x.astype(cnp, copy=False).reshape(NCH // 2, 2, P, D).transpose(0, 2, 1, 3)
        .reshape(NCH // 2, P, 2 * D)
    )

    def pack_sweeps(t):  # t: [n, i] transposed s-matrix -> [2, NB, P, SB*D]
        n = t.shape[0]
        a = t.reshape(n // (SB * P), SB, P, 2, D).transpose(3, 0, 2, 1, 4)
        return np.ascontiguousarray(a.reshape(2, n // (SB * P), P, SB * D))

    def pack_pairs(t):  # t: [n, i] -> [n/256, P, 2*i]
        n, i = t.shape
        a = t.reshape(n // (2 * P), 2, P, i).transpose(0, 2, 1, 3)
        return np.ascontiguousarray(a.reshape(n // (2 * P), P, 2 * i))

    in_maps = []
    for c in range(N_CORES):
        r0, r1 = ROWS * c, ROWS * (c + 1)
        # S = concat(s1, s3) rows; core c owns rows [r0, r1)
        if r1 <= K:
            s_rows = s1[r0:r1]
        elif r0 >= K:
            s_rows = s3[r0 - K : r1 - K]
        else:  # straddles the boundary (not the case for these shapes)
            s_rows = np.concatenate([s1[r0:], s3[: r1 - K]], axis=0)
        in_maps.append(
            {
                "xp": x_p,
                "w": w_p,
                "alpha": alpha,
                "bias": bias,
                "stp": pack_sweeps(prep(s_rows)),
                "s0p": pack_pairs(prep(s0[r0:r1])),
                "s2p": pack_pairs(prep(s2[r0:r1])),
            }
        )
    return in_maps


def kernel(x, weights, alpha, bias, s0, s1, s2, s3, _trace=False):
    from concourse.bass_utils import run_bass_kernel_spmd

    nc = _get_nc()
    in_maps = _shard_inputs(
        np.asarray(x), np.asarray(weights), np.asarray(alpha), np.asarray(bias),
        np.asarray(s0), np.asarray(s1), np.asarray(s2), np.asarray(s3),
    )
    kwargs = {}
    if _trace:
        # warm-up execution: compile + collective init + allocator warm so the
        # traced run measures steady-state
        run_bass_kernel_spmd(nc, in_maps, core_ids=list(range(N_CORES)))
        kwargs = dict(trace=True, trace_cores=list(range(N_CORES)))
    r = run_bass_kernel_spmd(nc, in_maps, core_ids=list(range(N_CORES)), **kwargs)
    full = np.concatenate([res["out"] for res in r.results], axis=0)
    if _trace:
        return full, r
    return full


# revision 22
# speedup vs baseline: 1.1652x; 1.0372x over previous
"""SpGAT_Conv Trainium2 kernel: 8-core SPMD spectral GNN conv.

Math (reference):
    a = softmax(alpha)
    pre = x @ W                                   [N, D]
    out_low  = s0 @ (a0 * (s1 @ pre))             [N, D]
    out_high = s2 @ (a1 * (s3 @ pre))             [N, D]
    out = relu(max(out_low, out_high) + bias)

Re-association: t = S @ (x @ W) == (S @ x) @ W with S = concat(s1, s3).
Row-sharding t's rows across 8 cores makes the x@W work perfectly sharded
too (it rides on each core's own 1024 rows of u = S_c @ x) instead of being
replicated, cutting per-core PE work from 1280 to 1056 big matmuls:

    step 1: u_c^T = x^T S_c^T accumulated over n-chunks; stationary = x
            chunks (natural layout), moving = S_c^T strips.  Two sweeps of
            512 i-columns each (4 PSUM banks per sweep).
    step 2: t_c = u_c @ W via u^T slices stationary, W moving (32 matmuls);
            each sweep's 512 t rows are staged + AllGathered in two
            sub-collectives (4 total, order 0,1,3,2) while later compute
            runs.
    phase 3: out_c = relu(max(a0*s0_c@t1 + bias, a1*s2_c@t3 + bias)), high
             band then low band over ARRIVAL-ordered t chunks; PSUM is
             pre-seeded with bias/a so no separate bias add exists in the
             epilogue.

DMA discipline: DMA_DIRECT2D executes synchronously on the issuing
engine's queue AND all queues share one small DMA-completion semaphore
pool, so a blocked DMA (or a semaphore-reset rendezvous with one) stalls
unrelated queues.  Mitigations:
  - Sync queue: only matmul-pacing strip loads, t_in stores, half the out
    stores — never anything collective-gated.
  - Activation queue: bulk x/w loads (head), AllGather-gated t_sb loads
    (tail), half the out stores.
  - Batched 3D-AP transfers minimize DMA instruction count (≈92 total) so
    semaphore epochs recycle rarely.
  - Vector/GpSimd split all PSUM drain/seed/stash/epilogue work.

All big operands are host-cast to bf16 (full PE rate) with fp32 PSUM
accumulation; s-matrices are host-transposed so the contraction dim lands
on SBUF partitions with contiguous DMAs.  x needs no transpose in this
formulation.
"""

import os

import numpy as np

N_CORES = 8
N = 8192
K = 2048
NK = N - K          # 6144
D = 512
ROWS = N // N_CORES  # 1024 rows per core
P = 128
RCH = ROWS // P      # 8  (row chunks per core / output strips)
NCH = N // P         # 64 (contraction chunks over full N)
KCH = K // P         # 16 (low-band chunks; high band = NCH - KCH = 48)
DCH = D // P         # 4  (depth chunks)
NSUB = 4             # sub-AllGathers for t
SUBR = ROWS // NSUB  # 256 rows per rank per sub-AG
SB = 4               # n-chunks per sweep-strip DMA batch

DEBUG = os.environ.get("SPGAT_DEBUG", "0") == "1"

_CACHE = {}

# t-chunk arrival order: sub-AG g delivers, for every rank c, t rows
# [1024c + 256g, 1024c + 256(g+1)) = global chunks 8c + 2g + {0,1}.
# Gathers are triggered in order 0, 1, 3, 2 (sweep B stages its upper rows
# first), so consume in that order too.  Phase 3 iterates PAIRS (both u's
# of one (g, c)) so each pair is one strip DMA.
GORDER = [0, 1, 3, 2]
PAIRS = [(8 * c + 2 * g, g, c) for g in GORDER for c in range(N_CORES)]


def _build_nc():
    import concourse.mybir as mybir
    import concourse.tile as tile
    from concourse import bacc

    f32 = mybir.dt.float32
    bf16 = mybir.dt.bfloat16
    cdt = bf16

    nc = bacc.Bacc(
        "TRN2", target_bir_lowering=False, debug=False, num_devices=N_CORES
    )

    # all big operands host-packed so every DMA reads 2KB+ contiguous
    # per-partition lines (max descriptor efficiency)
    xp = nc.dram_tensor("xp", [NCH // 2, P, 2 * D], cdt,
                        kind="ExternalInput").ap()
    w = nc.dram_tensor("w", [D, D], cdt, kind="ExternalInput").ap()
    alpha = nc.dram_tensor("alpha", [2], f32, kind="ExternalInput").ap()
    bias = nc.dram_tensor("bias", [D], f32, kind="ExternalInput").ap()
    stp = nc.dram_tensor("stp", [2, NCH // SB, P, SB * D], cdt,
                         kind="ExternalInput").ap()
    s0p = nc.dram_tensor("s0p", [KCH // 2, P, 2 * ROWS], cdt,
                         kind="ExternalInput").ap()
    s2p = nc.dram_tensor("s2p", [(NCH - KCH) // 2, P, 2 * ROWS], cdt,
                         kind="ExternalInput").ap()
    out = nc.dram_tensor("out", [ROWS, D], f32, kind="ExternalOutput").ap()
    if DEBUG:
        t_dump = nc.dram_tensor("t_dump", [N, D], cdt, kind="ExternalOutput").ap()

    groups = [list(range(N_CORES))]

    with tile.TileContext(nc) as tc:
        with (
            tc.tile_pool(name="const", bufs=1) as const,
            tc.tile_pool(name="bigA", bufs=1) as bigA,
            tc.tile_pool(name="bigB", bufs=1) as bigB,
            tc.tile_pool(name="strips1", bufs=4) as strips1,
            tc.tile_pool(name="strips3", bufs=3) as strips3,
            tc.tile_pool(name="stage", bufs=4) as stage,
            tc.tile_pool(name="epi", bufs=2) as epi,
            tc.tile_pool(name="stash", bufs=1) as stashp,
            tc.tile_pool(name="ps", bufs=8, space="PSUM") as ps,
            tc.tile_pool(name="dram", bufs=1, space="DRAM") as dram,
        ):
            # ---- collective warm-up: absorb first-collective init.  No
            # input deps so the trigger is the very first gpsimd instruction.
            warm_in = dram.tile([8, 8], f32, name="warm_in")
            warm_out = dram.tile([64, 8], f32, name="warm_out", addr_space="Shared")
            nc.gpsimd.collective_compute(
                "AllGather",
                mybir.AluOpType.bypass,
                replica_groups=groups,
                ins=[warm_in.opt()],
                outs=[warm_out.opt()],
            )

            # ---- PSUM allocation order fixes the 8-bank rotation; all
            # tiles are one 2KB bank.
            ps_a = ps.tile([P, 2], f32, name="ps_a", tag="acc")
            ps_b0 = ps.tile([P, D], f32, name="ps_b0", tag="acc")
            ps_b1 = ps.tile([P, D], f32, name="ps_b1", tag="acc")
            accsA = [ps.tile([P, D], f32, name=f"uA_{dc}", tag="acc")
                     for dc in range(DCH)]
            accsB = [ps.tile([P, D], f32, name=f"uB_{dc}", tag="acc")
                     for dc in range(DCH)]

            # ---- input DMAs: alpha/bias tiny on sync; x/w bulk on the
            # scalar (Activation) HWDGE queue so the sync queue only ever
            # carries matmul-pacing strip loads.
            asb = const.tile([1, 2], f32, name="asb")
            nc.sync.dma_start(asb[:], alpha[None, :])
            bsb = const.tile([1, D], f32, name="bsb")
            nc.sync.dma_start(bsb[:], bias[None, :])

            x_sb = bigA.tile([P, NCH, D], cdt, name="x_sb", tag="bigA")
            # block sizes (in 2-chunk units) pace delivery against sweep-A
            # consumption: small leading blocks so the first matmuls start
            # fast, growing blocks that always land before the PE needs them
            for b0, b1 in ((0, 1), (1, 2), (2, 3), (3, 4), (4, 6), (6, 8),
                           (8, 10), (10, 12), (12, 16), (16, 20), (20, 24),
                           (24, 32)):
                nc.scalar.dma_start(
                    x_sb[:, 2 * b0 : 2 * b1, :].rearrange("p c d -> p (c d)"),
                    xp[b0:b1].rearrange("b p a -> p b a"),
                )
            w_sb = const.tile([P, DCH, D], cdt, name="w_sb")
            nc.scalar.dma_start(w_sb[:], w.rearrange("(c p) d -> p c d", p=P))

            # ---- steps 1+2 infrastructure
            t_in = dram.tile([ROWS, D], cdt, name="t_in")
            t_outs = [
                dram.tile([SUBR * N_CORES, D], cdt, name=f"t_out{g}",
                          addr_space="Shared")
                for g in range(NSUB)
            ]

            def t_subag(g):
                nc.gpsimd.collective_compute(
                    "AllGather",
                    mybir.AluOpType.bypass,
                    replica_groups=groups,
                    ins=[t_in[SUBR * g : SUBR * (g + 1), :].opt()],
                    outs=[t_outs[g].opt()],
                )

            ut_sb = [
                const.tile([P, DCH, D], cdt, name=f"ut{sw}") for sw in range(2)
            ]

            def sweep1(sw, accs, b0, b1, split_first=False):
                """step-1 matmuls for strip batches [b0, b1) of sweep sw;
                each batch is SB n-chunks in one contiguous-packed DMA."""
                for bk in range(b0, b1):
                    j0 = SB * bk
                    strip = strips1.tile([P, SB, D], cdt, name=f"s{sw}_{bk}",
                                         tag="strip")
                    if split_first and bk == b0:
                        for j2 in range(SB):
                            nc.sync.dma_start(
                                strip[:, j2, :],
                                stp[sw, bk][:, D * j2 : D * (j2 + 1)],
                            )
                    else:
                        nc.sync.dma_start(
                            strip[:].rearrange("p a b -> p (a b)"),
                            stp[sw, bk],
                        )
                    for j2 in range(SB):
                        j = j0 + j2
                        for dc in range(DCH):
                            nc.tensor.matmul(
                                accs[dc][:],
                                x_sb[:, j, P * dc : P * (dc + 1)],
                                strip[:, j2, :],
                                start=(j == 0),
                                stop=(j == NCH - 1),
                            )

            def drain_u(sw, accs):
                for dc in range(DCH):
                    nc.vector.tensor_copy(ut_sb[sw][:, dc, :], accs[dc][:])

            def step2(sw, tps=None):
                """t rows [512sw, 512sw+512): 4 i-blocks of 128 rows; stage
                + trigger this sweep's two sub-AGs (B stages upper first)."""
                ib_order = [0, 1, 2, 3] if sw == 0 else [2, 3, 0, 1]
                ag_map = {1: 0, 3: 1} if sw == 0 else {1: 3, 3: 2}
                for i, ib in enumerate(ib_order):
                    if tps is None:
                        tp = ps.tile([P, D], f32, name=f"t_ps_{sw}_{ib}",
                                     tag="acc")
                    else:
                        tp = tps[i]
                    for dc in range(DCH):
                        nc.tensor.matmul(
                            tp[:],
                            ut_sb[sw][:, dc, P * ib : P * (ib + 1)],
                            w_sb[:, dc, :],
                            start=(dc == 0),
                            stop=(dc == DCH - 1),
                        )
                    tst = stage.tile([P, D], cdt, name=f"t_st_{sw}_{ib}",
                                     tag="st")
                    nc.vector.tensor_copy(tst[:], tp[:])
                    row0 = D * sw + P * ib
                    nc.sync.dma_start(t_in[row0 : row0 + P, :], tst[:])
                    if i in ag_map:
                        t_subag(ag_map[i])

            NB = NCH // SB  # 16 strip batches per sweep

            # ---- PE stream head: first strip batch goes before the softmax
            # setup matmuls so the PE starts as soon as chunk 0 lands.
            sweep1(0, accsA, 0, 1, split_first=True)

            # softmax(alpha) chain (vector) + broadcasts via tiny matmuls
            amax = const.tile([1, 1], f32, name="amax")
            nc.vector.tensor_tensor(
                amax[:], asb[:, 0:1], asb[:, 1:2], mybir.AluOpType.max
            )
            ash = const.tile([1, 2], f32, name="ash")
            nc.vector.tensor_scalar(
                ash[:], asb[:], amax[:, 0:1], None, mybir.AluOpType.subtract
            )
            aexp = const.tile([1, 2], f32, name="aexp")
            nc.scalar.activation(aexp[:], ash[:], mybir.ActivationFunctionType.Exp)
            asum = const.tile([1, 1], f32, name="asum")
            nc.vector.tensor_tensor(
                asum[:], aexp[:, 0:1], aexp[:, 1:2], mybir.AluOpType.add
            )
            arec = const.tile([1, 1], f32, name="arec")
            nc.vector.reciprocal(arec[:], asum[:])
            afin = const.tile([1, 2], f32, name="afin")
            nc.vector.tensor_scalar(
                afin[:], aexp[:], arec[:, 0:1], None, mybir.AluOpType.mult
            )
            ainv = const.tile([1, 2], f32, name="ainv")
            nc.vector.reciprocal(ainv[:], afin[:])
            srow0 = const.tile([1, D], f32, name="srow0")
            nc.vector.tensor_scalar(
                srow0[:], bsb[:], ainv[:, 0:1], None, mybir.AluOpType.mult
            )
            srow1 = const.tile([1, D], f32, name="srow1")
            nc.vector.tensor_scalar(
                srow1[:], bsb[:], ainv[:, 1:2], None, mybir.AluOpType.mult
            )
            ones = const.tile([1, P], f32, name="ones")
            nc.vector.memset(ones[:], 1.0)
            nc.tensor.matmul(ps_a[:], ones[:], afin[:], start=True, stop=True)
            a128 = const.tile([P, 2], f32, name="a128")
            nc.vector.tensor_copy(a128[:], ps_a[:])
            seeds = const.tile([P, 2 * D], f32, name="seeds")
            nc.tensor.matmul(ps_b0[:], ones[:], srow0[:], start=True, stop=True)
            nc.vector.tensor_copy(seeds[:, 0:D], ps_b0[:])
            nc.tensor.matmul(ps_b1[:], ones[:], srow1[:], start=True, stop=True)
            nc.vector.tensor_copy(seeds[:, D : 2 * D], ps_b1[:])
            seed_lo = seeds[:, 0:D]
            seed_hi = seeds[:, D : 2 * D]

            # ---- rest of sweep A, then pipelined B/step2
            sweep1(0, accsA, 1, NB)
            drain_u(0, accsA)
            # head of sweep B hides sweep A's drain latency before step2A
            sweep1(1, accsB, 0, 2)
            step2(0)
            sweep1(1, accsB, 2, NB)
            drain_u(1, accsB)

            # PSUM rotation: allocate step2B's tiles, then phase-3
            # accumulators — accs3[0..3] land on step2A's banks (free
            # early), accs3[4..7] on step2B's.  Seed 0..3 before step2B's
            # instructions so only 4..7 wait on its staging.
            tps1 = [ps.tile([P, D], f32, name=f"t_ps_1_{i}", tag="acc")
                    for i in range(4)]
            accs3 = [
                ps.tile([P, D], f32, name=f"acc3_{nt}", tag="acc")
                for nt in range(RCH)
            ]
            for nt in range(4):
                nc.vector.tensor_copy(accs3[nt][:], seed_hi)
            step2(1, tps1)
            for nt in range(4, RCH):
                nc.scalar.copy(accs3[nt][:], seed_hi)

            # ---- phase 3: t gather consumption + band matmuls.  One t_sb
            # load per (sub-AG, u): 8 gather DMAs total on the scalar queue.
            # t_sb loads ride the otherwise-idle gpsimd queue: their
            # AllGather-gated waits then cannot rendezvous-block the two
            # HWDGE queues through the shared semaphore pool.
            t_sb = bigB.tile([P, NCH, D], cdt, name="t_sb", tag="bigB")
            t_sb_r = t_sb[:].rearrange("p (c r) d -> p r c d", r=8)
            for g in GORDER:
                for u in range(2):
                    for h in range(2):
                        # chunks j = 8c + 2g + u, half-batches so the first
                        # chunks land sooner after the gather completes
                        nc.gpsimd.dma_start(
                            t_sb_r[:, 2 * g + u, 4 * h : 4 * h + 4, :],
                            t_outs[g].rearrange(
                                "(c q p) d -> p c q d", p=P, q=2
                            )[:, 4 * h : 4 * h + 4, u, :],
                        )
            if DEBUG:
                for j in range(NCH):
                    nc.sync.dma_start(
                        t_dump[P * j : P * (j + 1), :], t_sb[:, j, :]
                    )

            HI_PAIRS = [e for e in PAIRS if e[0] >= KCH]
            LO_PAIRS = [e for e in PAIRS if e[0] < KCH]
            stash = [
                stashp.tile([P, D], f32, name=f"hst_{nt}", tag=f"hst{nt}")
                for nt in range(RCH)
            ]
            for idx, (j, g, c) in enumerate(HI_PAIRS):
                jj = j - KCH
                strip = strips3.tile([P, 2, ROWS], cdt, name=f"rh_{j}",
                                     tag="strip3")
                nc.sync.dma_start(
                    strip[:].rearrange("p a b -> p (a b)"), s2p[jj // 2]
                )
                for u in range(2):
                    for nt in range(RCH):
                        nc.tensor.matmul(
                            accs3[nt][:],
                            strip[:, u, P * nt : P * (nt + 1)],
                            t_sb[:, j + u, :],
                            start=False,
                            stop=(idx == len(HI_PAIRS) - 1 and u == 1),
                        )
            # stash = a1*acc = a1*hi + bias, then re-seed for the low band
            # right behind the stash read; split vector/scalar (gpsimd
            # cannot access PSUM)
            for nt in range(4):
                nc.vector.tensor_scalar(
                    stash[nt][:], accs3[nt][:], a128[:, 1:2], None,
                    mybir.AluOpType.mult,
                )
                nc.vector.tensor_copy(accs3[nt][:], seed_lo)
            for nt in range(4, RCH):
                nc.scalar.mul(stash[nt][:], accs3[nt][:], a128[:, 1:2])
                nc.scalar.copy(accs3[nt][:], seed_lo)
            # relu the stash in place (off the critical path, during the low
            # band): relu(max(u,v)) == max(relu(u), relu(v)) lets the final
            # per-strip chain be one ACT op + one DVE max
            for nt in range(RCH):
                nc.scalar.activation(
                    stash[nt][:], stash[nt][:],
                    mybir.ActivationFunctionType.Relu,
                )
            for idx, (j, g, c) in enumerate(LO_PAIRS):
                strip = strips3.tile([P, 2, ROWS], cdt, name=f"rl_{j}",
                                     tag="strip3")
                nc.sync.dma_start(
                    strip[:].rearrange("p a b -> p (a b)"), s0p[j // 2]
                )
                for u in range(2):
                    for nt in range(RCH):
                        nc.tensor.matmul(
                            accs3[nt][:],
                            strip[:, u, P * nt : P * (nt + 1)],
                            t_sb[:, j + u, :],
                            start=False,
                            stop=(idx == len(LO_PAIRS) - 1 and u == 1),
                        )
            for nt in range(RCH):
                lo = epi.tile([P, D], f32, name=f"elo_{nt}", tag="elo")
                # relu(a0*acc) straight out of PSUM (bias already seeded in)
                nc.scalar.activation(
                    lo[:], accs3[nt][:], mybir.ActivationFunctionType.Relu,
                    scale=a128[:, 0:1],
                )
                osb = epi.tile([P, D], f32, name=f"osb_{nt}", tag="osb")
                nc.vector.tensor_tensor(
                    osb[:], lo[:], stash[nt][:], mybir.AluOpType.max
                )
                row0 = P * nt
                nc.sync.dma_start(out[row0 : row0 + P, :], osb[:])

    nc.compile()
    return nc


def _get_nc():
    if "nc" not in _CACHE:
        _CACHE["nc"] = _build_nc()
    return _CACHE["nc"]


def _shard_inputs(x, weights, alpha, bias, s0, s1, s2, s3):
    import ml_dtypes

    cnp = ml_dtypes.bfloat16

    def prep(a):  # transpose + cast, C-contiguous
        return np.ascontiguousarray(a.T).astype(cnp, copy=False)

    alpha = np.ascontiguousarray(alpha, dtype=np.float32)
    bias = np.ascontiguousarray(bias, dtype=np.float32)
    w_p = np.ascontiguousarray(weights).astype(cnp, copy=False)
    # xp[b, p, (j2 d)] = x[128*(2b+j2)+p, d]: 2KB contiguous per partition
    x_p = np.ascontiguousarray(
        x.astype(cnp, copy=False).reshape(NCH // 2, 2, P, D).transpose(0, 2, 1, 3)
        .reshape(NCH // 2, P, 2 * D)
    )

    def pack_sweeps(t):  # t: [n, i] transposed s-matrix -> [2, NB, P, SB*D]
        n = t.shape[0]
        a = t.reshape(n // (SB * P), SB, P, 2, D).transpose(3, 0, 2, 1, 4)
        return np.ascontiguousarray(a.reshape(2, n // (SB * P), P, SB * D))

    def pack_pairs(t):  # t: [n, i] -> [n/256, P, 2*i]
        n, i = t.shape
        a = t.reshape(n // (2 * P), 2, P, i).transpose(0, 2, 1, 3)
        return np.ascontiguousarray(a.reshape(n // (2 * P), P, 2 * i))

    in_maps = []
    for c in range(N_CORES):
        r0, r1 = ROWS * c, ROWS * (c + 1)
        # S = concat(s1, s3) rows; core c owns rows [r0, r1)
        if r1 <= K:
            s_rows = s1[r0:r1]
        elif r0 >= K:
            s_rows = s3[r0 - K : r1 - K]
        else:  # straddles the boundary (not the case for these shapes)
            s_rows = np.concatenate([s1[r0:], s3[: r1 - K]], axis=0)
        in_maps.append(
            {
                "xp": x_p,
                "w": w_p,
                "alpha": alpha,
                "bias": bias,
                "stp": pack_sweeps(prep(s_rows)),
                "s0p": pack_pairs(prep(s0[r0:r1])),
                "s2p": pack_pairs(prep(s2[r0:r1])),
            }
        )
    return in_maps


def kernel(x, weights, alpha, bias, s0, s1, s2, s3, _trace=False):
    from concourse.bass_utils import run_bass_kernel_spmd

    nc = _get_nc()
    in_maps = _shard_inputs(
        np.asarray(x), np.asarray(weights), np.asarray(alpha), np.asarray(bias),
        np.asarray(s0), np.asarray(s1), np.asarray(s2), np.asarray(s3),
    )
    kwargs = {}
    if _trace:
        # warm-up execution: compile + collective init + allocator warm so the
        # traced run measures steady-state
        run_bass_kernel_spmd(nc, in_maps, core_ids=list(range(N_CORES)))
        kwargs = dict(trace=True, trace_cores=list(range(N_CORES)))
    r = run_bass_kernel_spmd(nc, in_maps, core_ids=list(range(N_CORES)), **kwargs)
    full = np.concatenate([res["out"] for res in r.results], axis=0)
    if _trace:
        return full, r
    return full
